# revision 37
# baseline (speedup 1.0000x reference)
"""GCN encoder kernel for 8 Trainium2 NeuronCores (Bass/Tile, SPMD).

Strategy (dst-sharded graph parallel):
  - Nodes degree-sorted, padded to NPAD = 392 tiles of 128; tiles round-robin
    across the 8 cores (SPMD: one program, 8 in_maps).
  - Layer fusion: aggregation is linear, so each GCN layer is (aggregate)@W.T;
    layers 1 and 2 share ONE aggregation of h.
  - agg0 (over x): host expands x*dinv[src] into a padded per-(node,slot)
    fp8e3 stream (rms-normalized; the scale is folded into W0); device
    reduces with PE matmuls against an fp8 identity (PSUM scatter-add).
  - h is exchanged in FOUR AllGather slices (A1/A2 cover the first KA=18
    per-core tiles -> h_fullA, B1/B2 the last 31 -> h_fullB), each issued as
    soon as its h tiles exist, so agg1 pass 1 starts ~100us in, while phase A
    is still running.  Both gathered tensors have <32768 rows so int16
    gather indices need no range splitting.
  - agg1 (over h) runs as TWO PASSES over all tiles: pass 1 accumulates the
    h_fullA-half of every tile into f32 SBUF partials and is INTERLEAVED
    into the phase-A loop tail (PE alternates phase-A and pass-1 matmuls,
    and the serial SWDGE gather stream starts early); pass 2 adds the
    h_fullB half + the self-loop (one matmul against the SBUF-resident
    local h tile -- no gather) + partial, then applies W1/W2 and writes
    outputs.  Pass-1 gathers are pair-batched (2 tiles per SWDGE call) to
    amortize the ~1us fixed Q7 cost per dma_gather.
  - One-hot S matrices are built on-chip by DVE is_equal against a resident
    iota tile; the dst-row stream is PAIR-DUPLICATED (dk2) so every operand's
    innermost dim is packed stride-1 and the DVE 2x mode engages.
  - Weight matmuls run in f16; symmetric normalization is folded into
    host-side scales; outputs written f16 and upcast on host.
"""
import os
import sys

sys.path.insert(0, "/opt/trn_rl_repo")

import numpy as np

N, E, DIN, DH = 50000, 1600000, 128, 128
NCORES = 8
NPAD = ((N + 1023) // 1024) * 1024   # 50176 = 392 tiles of 128
TILES = NPAD // 128
TPC = TILES // NCORES                # positions (tiles) per core
NPC = TPC * 128                      # node rows per core
KA = 18                              # tiles in first AllGather half
KB = TPC - KA                        # tiles in second half (31)
NRA = NCORES * KA * 128              # 18432 rows in h_fullA (< 32768)
NRB = NCORES * KB * 128              # 31744 rows in h_fullB (< 32768)


def _wrap_idx16(a):
    """dma_gather index layout: idx i -> [i%16, i//16], replicated 8x."""
    n = len(a)
    w = np.zeros((16, n // 16), np.int16)
    w[np.arange(n) % 16, np.arange(n) // 16] = a
    return np.tile(w, (8, 1))


def _build_kernel(CCA, CCB, CA, has_b0, has_b12):
    """Build the SPMD Tile program. CCA/CCB/CA are per-position chunk
    counts (compile-time constants, shared by all cores)."""
    import concourse.bass as bass  # noqa: F401
    import concourse.tile as tile
    from concourse import bacc, mybir

    f32, f16, i16 = mybir.dt.float32, mybir.dt.float16, mybir.dt.int16
    f8 = mybir.dt.float8e4
    SA, SB = sum(CCA), sum(CCB)
    SCA = sum(CA)
    CAmax = max(CA)
    # gathers are pair-batched: one SWDGE call covers tiles (2j, 2j+1)
    PAIRS = [(2 * j, 2 * j + 1) if 2 * j + 1 < TPC else (2 * j,)
             for j in range((TPC + 1) // 2)]
    GW = max(max(sum(CCA[t] for t in pr) for pr in PAIRS),
             max(sum(CCB[t] for t in pr) for pr in PAIRS))
    CCmax = max(max(CCA), max(CCB))

    nc = bacc.Bacc(None, target_bir_lowering=False, debug=False,
                   num_swdge_queues=4)

    xe_d = nc.dram_tensor("xe", [128, SCA * 128], f8, kind="ExternalInput")
    m1_d = nc.dram_tensor("m1", [TPC, 128, 128], f16, kind="ExternalInput")
    ia_d = nc.dram_tensor("ia", [128, SA * 8], i16, kind="ExternalInput")
    ib_d = nc.dram_tensor("ib", [128, SB * 8], i16, kind="ExternalInput")
    dk_d = nc.dram_tensor("dk2", [128, 2 * (SA + SB)], f16,
                          kind="ExternalInput")
    dinv_d = nc.dram_tensor("dinvp", [128, TPC], f32, kind="ExternalInput")
    ident_d = nc.dram_tensor("ident", [128, 128], f16, kind="ExternalInput")
    ident8_d = nc.dram_tensor("ident82", [128, 256], f8, kind="ExternalInput")
    iota_d = nc.dram_tensor("iota", [128, 128], f16, kind="ExternalInput")
    w0_d = nc.dram_tensor("w0t", [128, 128], f16, kind="ExternalInput")
    w1_d = nc.dram_tensor("w1t", [128, 128], f16, kind="ExternalInput")
    w2_d = nc.dram_tensor("w2t", [128, 128], f16, kind="ExternalInput")
    if has_b0:
        m2_d = nc.dram_tensor("m2", [TPC, 128, 128], f16, kind="ExternalInput")
    if has_b12:
        b1_d = nc.dram_tensor("b1b", [128, 128], f32, kind="ExternalInput")
        b2_d = nc.dram_tensor("b2b", [128, 128], f32, kind="ExternalInput")
    o1_d = nc.dram_tensor("o1", [TPC, 128, 128], f16, kind="ExternalOutput")
    o2_d = nc.dram_tensor("o2", [TPC, 128, 128], f16, kind="ExternalOutput")

    offA = np.cumsum([0] + CCA).tolist()
    offB = np.cumsum([0] + CCB).tolist()
    GRP = [list(range(NCORES))]

    with tile.TileContext(nc) as tc:
        with (
            tc.tile_pool(name="const", bufs=1) as cpool,
            tc.tile_pool(name="acc", bufs=1) as apool,
            tc.tile_pool(name="dram", bufs=1, space="DRAM") as dpool,
            tc.tile_pool(name="g", bufs=5) as gpool,
            tc.tile_pool(name="s", bufs=2) as spool,
            tc.tile_pool(name="y2", bufs=2) as ypool2,
            tc.tile_pool(name="o", bufs=3) as opool,
            tc.tile_pool(name="psb", bufs=2, space="PSUM") as ppoolb,
            tc.tile_pool(name="psb2", bufs=1, space="PSUM") as ppoolb2,
        ):
            ident_sb = cpool.tile([128, 128], f16)
            nc.scalar.dma_start(ident_sb[:], ident_d[:])
            ident8_sb = cpool.tile([128, 256], f8)
            nc.scalar.dma_start(ident8_sb[:], ident8_d[:])
            iota_sb = cpool.tile([128, 128], f16)
            nc.scalar.dma_start(iota_sb[:], iota_d[:])
            w0_sb = cpool.tile([128, 128], f16)
            nc.scalar.dma_start(w0_sb[:], w0_d[:])
            w1_sb = cpool.tile([128, 128], f16)
            nc.scalar.dma_start(w1_sb[:], w1_d[:])
            w2_sb = cpool.tile([128, 128], f16)
            nc.scalar.dma_start(w2_sb[:], w2_d[:])
            dinv_sb = cpool.tile([128, TPC], f32)
            nc.scalar.dma_start(dinv_sb[:], dinv_d[:])
            ia_sb = cpool.tile([128, SA * 8], i16)
            ib_sb = cpool.tile([128, SB * 8], i16)
            dk_sb = cpool.tile([128, 2 * (SA + SB)], f16)
            if has_b12:
                b1_sb = cpool.tile([128, 128], f32)
                nc.scalar.dma_start(b1_sb[:], b1_d[:])
                b2_sb = cpool.tile([128, 128], f32)
                nc.scalar.dma_start(b2_sb[:], b2_d[:])

            h_sbA = apool.tile([128, KA, 128], f16)
            h_sbB = apool.tile([128, KB, 128], f16)
            part_sb = apool.tile([128, TPC, 128], f16)
            m1_sb = apool.tile([128, TPC, 128], f16)
            nc.scalar.dma_start(
                m1_sb[:], m1_d[:].rearrange("t p f -> p t f"))

            h_locA = dpool.tile([KA, 128, 128], f16)
            h_locB = dpool.tile([KB, 128, 128], f16)
            h_fullA = dpool.tile([NRA, 128], f16, addr_space="Shared")
            h_fullB = dpool.tile([NRB, 128], f16, addr_space="Shared")

            qctr = [0]

            def gather_pair(j, which):
                pr = PAIRS[j]
                CC = CCA if which == 0 else CCB
                cc = sum(CC[t] for t in pr)
                off = (offA if which == 0 else offB)[pr[0]]
                src = h_fullA if which == 0 else h_fullB
                idx = ia_sb if which == 0 else ib_sb
                G = gpool.tile([128, GW, 128], f16, tag="G")
                nc.gpsimd.dma_gather(G[:, 0:cc, :], src[:],
                                     idx[:, off * 8:(off + cc) * 8],
                                     cc * 128, cc * 128, 128,
                                     elem_step=128, single_packet=False,
                                     queue_num=qctr[0] % 4)
                qctr[0] += 1
                return G

            def build_S(k, which):
                cc = (CCA if which == 0 else CCB)[k]
                col = (offA[k] if which == 0 else SA + offB[k])
                S = spool.tile([128, CCmax, 128], f16, tag="S")
                iota4 = iota_sb[:].rearrange(
                    "p (o f g) -> p o f g", o=1, g=2).broadcast_to(
                    [128, cc, 64, 2])
                dk4 = dk_sb[:, 2 * col:2 * (col + cc)].rearrange(
                    "p (c o g) -> p c o g", o=1, g=2).broadcast_to(
                    [128, cc, 64, 2])
                S4 = S[:, 0:cc, :].rearrange("p c (f g) -> p c f g", g=2)
                nc.vector.tensor_tensor(S4, iota4, dk4,
                                        mybir.AluOpType.is_equal)
                return S

            PRE1 = 2                 # prefetched pass-1 pair gathers
            PRE2 = 2                 # prefetched pass-2 pair gathers
            BCOLL_AT = 4             # pass-1 pair after which B-coll is issued
            Gp = [None] * len(PAIRS)
            np_ctr = [0]

            def process_pair(j):
                if j + PRE1 < len(PAIRS):
                    Gp[j + PRE1] = gather_pair(j + PRE1, 0)
                G = Gp[j]
                Gp[j] = None
                goff = 0
                for t in PAIRS[j]:
                    cca = CCA[t]
                    S = build_S(t, 0)
                    ps_b = ppoolb.tile([128, 128], f32, tag="pb")
                    for c in range(cca):
                        nc.tensor.matmul(ps_b[:], G[:, goff + c, :],
                                         S[:, c, :],
                                         start=(c == 0), stop=(c == cca - 1))
                    nc.scalar.copy(part_sb[:, t, :], ps_b[:])
                    goff += cca

            # ---------------- Phase A: agg0 + h (pass 1 interleaved) ------
            with (
                tc.tile_pool(name="xe", bufs=3) as xpool,
                tc.tile_pool(name="meta", bufs=3) as mpool,
                tc.tile_pool(name="y", bufs=3) as ypool,
                tc.tile_pool(name="ps", bufs=3, space="PSUM") as ppool,
                tc.tile_pool(name="ps2", bufs=2, space="PSUM") as ppool2,
            ):
                # the W0 matmul of tile k-1 is emitted AFTER tile k's psum
                # accumulation: the PE then never waits on the scalar copy,
                # stays continuously busy, and ramps to the full pstate.
                def w0_tail(k, y_sb):
                    ps_h = ppool2.tile([128, 128], f32, tag="ph")
                    nc.tensor.matmul(ps_h[:], y_sb[:], w0_sb[:],
                                     start=True, stop=True)  # [n, f2]
                    tmp = ypool.tile([128, 128], f32, tag="tmp")
                    nc.vector.tensor_tensor(tmp[:], ps_h[:], m1_sb[:, k, :],
                                            mybir.AluOpType.mult)
                    if has_b0:
                        m2_sb = mpool.tile([128, 128], f16, tag="m2")
                        nc.sync.dma_start(m2_sb[:], m2_d[k])
                        nc.vector.tensor_tensor(tmp[:], tmp[:], m2_sb[:],
                                                mybir.AluOpType.add)
                    if k < KA:
                        nc.scalar.activation(h_sbA[:, k, :], tmp[:],
                                             mybir.ActivationFunctionType.Relu)
                    else:
                        nc.scalar.activation(h_sbB[:, k - KA, :], tmp[:],
                                             mybir.ActivationFunctionType.Relu)

                acol = 0
                pend = [None, None]          # (k, y_sb) awaiting its W0 tail
                for k in range(TPC):
                    ca = CA[k]               # even by construction
                    ca2 = ca // 2
                    gt = xpool.tile([128, CAmax // 2, 256], f8, tag="gt")
                    nc.sync.dma_start(
                        gt[:, 0:ca2, :].rearrange("p c f -> p (c f)"),
                        xe_d[:, acol * 128:(acol + ca) * 128])
                    acol += ca

                    # fp8e4 DoubleRow: each matmul contracts TWO slots
                    # (lhsT = [slot2j | slot2j+1], rhs = [I | I]), halving
                    # the PE instruction count -- phase A is issue-bound.
                    ps_a = ppool.tile([128, 128], f32, tag="pa")
                    id2 = ident8_sb[:].rearrange("p (two f) -> p two f", two=2)
                    for j in range(ca2):
                        nc.tensor.matmul(ps_a[:],
                                         gt[:, j, :].rearrange(
                                             "p (two f) -> p two f", two=2),
                                         id2,
                                         start=(j == 0), stop=(j == ca2 - 1),
                                         perf_mode=mybir.MatmulPerfMode.DoubleRow)
                    y_sb = ypool.tile([128, 128], f16, tag="y0")
                    nc.scalar.copy(y_sb[:], ps_a[:])        # [f, d] raw sums
                    if pend[0] is not None:
                        w0_tail(*pend)
                    pend = [k, y_sb]

                    if k == 1:
                        nc.scalar.dma_start(ia_sb[:], ia_d[:])
                        nc.scalar.dma_start(ib_sb[:], ib_d[:])
                        nc.scalar.dma_start(dk_sb[:], dk_d[:])
                    if k == KA:
                        # tile KA-1's tail ran above, so h_sbA is complete
                        nc.gpsimd.dma_start(
                            h_locA[:].rearrange("t p f -> p t f"), h_sbA[:])
                        nc.gpsimd.collective_compute(
                            "AllGather", mybir.AluOpType.bypass,
                            replica_groups=GRP,
                            ins=[h_locA[:]], outs=[h_fullA[:]],
                        )
                w0_tail(*pend)
                # gathers start only now: their transfers would otherwise
                # starve the phase-A xe stream on the shared DMA engines and
                # stall the h tail (which gates the B collective).
                for jj in range(PRE1):
                    Gp[jj] = gather_pair(jj, 0)
                while np_ctr[0] < min(BCOLL_AT + 1, len(PAIRS)):
                    process_pair(np_ctr[0])
                    np_ctr[0] += 1
                # all h writes are emitted now; Pool reaches this point at
                # ~pair-14 time, long after the h tiles are computed, so the
                # collective dispatches without blocking the gather stream.
                # Pool issues the DMA (Sync would deadlock behind gt loads).
                nc.gpsimd.dma_start(
                    h_locB[:].rearrange("t p f -> p t f"), h_sbB[:])
                nc.gpsimd.collective_compute(
                    "AllGather", mybir.AluOpType.bypass,
                    replica_groups=GRP,
                    ins=[h_locB[:]], outs=[h_fullB[:]],
                )
                while np_ctr[0] < len(PAIRS):
                    process_pair(np_ctr[0])
                    np_ctr[0] += 1

            # ---------------- Phase B pass 2: B-half + outputs ------------
            Gp2 = [None] * len(PAIRS)
            for jj in range(min(PRE2, len(PAIRS))):
                Gp2[jj] = gather_pair(jj, 1)
            for j in range(len(PAIRS)):
                if j + PRE2 < len(PAIRS):
                    Gp2[j + PRE2] = gather_pair(j + PRE2, 1)
                G = Gp2[j]
                Gp2[j] = None
                goff = 0
                for k in PAIRS[j]:
                    ccb = CCB[k]
                    S = build_S(k, 1)
                    h_self = (h_sbA[:, k, :] if k < KA
                              else h_sbB[:, k - KA, :])
                    ps_b = ppoolb.tile([128, 128], f32, tag="pb")
                    # self-loop: h of this tile is SBUF-resident, no gather
                    nc.tensor.matmul(ps_b[:], h_self, ident_sb[:],
                                     start=True, stop=False)
                    for c in range(ccb):
                        nc.tensor.matmul(ps_b[:], G[:, goff + c, :],
                                         S[:, c, :],
                                         start=False, stop=(c == ccb - 1))
                    goff += ccb
                    y2 = ypool2.tile([128, 128], f16, tag="y2")
                    nc.vector.tensor_tensor(y2[:], ps_b[:], part_sb[:, k, :],
                                            mybir.AluOpType.add)

                    ps_o1 = ppoolb2.tile([128, 128], f32, tag="po")
                    nc.tensor.matmul(ps_o1[:], y2[:], w1_sb[:],
                                     start=True, stop=True)
                    o1t = opool.tile([128, 128], f16, tag="o1")
                    nc.scalar.activation(o1t[:], ps_o1[:],
                                         mybir.ActivationFunctionType.Copy,
                                         scale=dinv_sb[:, k:k + 1])
                    ps_o2 = ppoolb2.tile([128, 128], f32, tag="po")
                    nc.tensor.matmul(ps_o2[:], y2[:], w2_sb[:],
                                     start=True, stop=True)
                    o2t = opool.tile([128, 128], f16, tag="o2")
                    nc.scalar.activation(o2t[:], ps_o2[:],
                                         mybir.ActivationFunctionType.Copy,
                                         scale=dinv_sb[:, k:k + 1])
                    if has_b12:
                        nc.vector.tensor_tensor(o1t[:], o1t[:], b1_sb[:],
                                                mybir.AluOpType.add)
                        nc.vector.tensor_tensor(o2t[:], o2t[:], b2_sb[:],
                                                mybir.AluOpType.add)
                    nc.sync.dma_start(o1_d[k], o1t[:])
                    nc.sync.dma_start(o2_d[k], o2t[:])

    nc.compile()
    return nc


def kernel(x, edge_index, drop_mask, W0, b0, W1, b1, W2, b2, **_):
    import ml_dtypes
    from concourse.bass_utils import run_bass_kernel_spmd

    x = np.asarray(x, np.float32)
    edge_index = np.asarray(edge_index)
    drop_mask = np.asarray(drop_mask, np.float32)
    W0, W1, W2 = (np.asarray(w, np.float32) for w in (W0, W1, W2))
    b0, b1, b2 = (np.asarray(b, np.float32) for b in (b0, b1, b2))
    src0, dst0 = edge_index[0].astype(np.int64), edge_index[1].astype(np.int64)

    # ---- normalization / permutation (host: index-side preprocessing) ----
    deg = np.bincount(dst0, minlength=N).astype(np.float32) + 1.0
    dinv = 1.0 / np.sqrt(deg)

    perm = np.argsort(-deg, kind="stable")           # position -> node id
    pos = np.empty(N, np.int64)                      # node id -> position
    pos[perm] = np.arange(N)

    # self loops as ordinary edges; the appended ones (apnd) are handled in
    # phase B by a local matmul instead of gathers.
    src_a = np.concatenate([src0, np.arange(N)])
    dst_a = np.concatenate([dst0, np.arange(N)])
    apnd = np.concatenate([np.zeros(E, bool), np.ones(N, bool)])
    sp = pos[src_a]
    dp = pos[dst_a]

    # h storage row of a source position, split across the two AllGathers:
    #   tile t = p//128 -> core t%8, per-core tile index t//8
    #   first KA per-core tiles -> h_fullA, rest -> h_fullB
    st = sp // 128
    sk = st // NCORES
    in_a = sk < KA
    hrow = np.where(
        in_a,
        (st % NCORES) * (KA * 128) + sk * 128 + (sp % 128),
        (st % NCORES) * (KB * 128) + (sk - KA) * 128 + (sp % 128))

    tile_of = dp // 128
    core_of = tile_of % NCORES
    kpos_of = tile_of // NCORES

    order = np.lexsort((sp, dp))
    sp, dp = sp[order], dp[order]
    core_of, kpos_of = core_of[order], kpos_of[order]
    hrow, in_a, apnd = hrow[order], in_a[order], apnd[order]
    dloc = dp % 128

    # fp8e3 x-stream: rms-normalize so values sit in e3m4's sweet spot;
    # the scale is undone inside W0 (aggregation is linear).
    x_pre = x * dinv[:, None]
    sx = float(1.0 / np.sqrt((x_pre ** 2).mean()))
    x_pre_pos = np.zeros((NPAD + 1, 128), ml_dtypes.float8_e4m3)
    x_pre_pos[pos] = (x_pre * sx).astype(ml_dtypes.float8_e4m3)
    dinv_pos = np.zeros(NPAD, np.float32)
    dinv_pos[pos] = dinv

    # ---- per-(core, position) edge groups ----
    EB = [[None] * TPC for _ in range(NCORES)]
    for c in range(NCORES):
        mc = core_of == c
        spc, kc, dl = sp[mc], kpos_of[mc], dloc[mc]
        hr, ia, ap_ = hrow[mc], in_a[mc], apnd[mc]
        for k in range(TPC):
            mk = kc == k
            EB[c][k] = (hr[mk], dl[mk], spc[mk], ia[mk], ap_[mk])

    # per-position chunk counts (max over cores -> same program everywhere)
    # CA: phase-A slots (all edges incl. appended self loops)
    # CCA/CCB: phase-B gather chunks (appended self loops excluded)
    CCA, CCB, CA = [], [], []
    for k in range(TPC):
        cca = ccb = ca = 0
        for c in range(NCORES):
            hr, dl, _, ia, ap_ = EB[c][k]
            ns = ~ap_
            na = int((ia & ns).sum())
            nb = int((~ia & ns).sum())
            cca = max(cca, -(-na // 128))
            ccb = max(ccb, -(-nb // 128))
            if len(dl):
                ca = max(ca, int(np.bincount(dl, minlength=128).max()))
        CCA.append(max(cca, 1))
        CCB.append(max(ccb, 1))
        ca = max(ca, 2)
        CA.append(ca + (ca & 1))             # even, for DoubleRow pairs
    SCA, SA, SB = sum(CA), sum(CCA), sum(CCB)

    ident_np = np.eye(128, dtype=np.float16)
    ident8_np = np.tile(np.eye(128), (1, 2)).astype(ml_dtypes.float8_e4m3)
    iota_np = np.tile(np.arange(128, dtype=np.float16), (128, 1))
    has_b0 = bool(np.any(b0))
    has_b12 = bool(np.any(b1)) or bool(np.any(b2))

    in_maps = []
    for c in range(NCORES):
        xe = np.zeros((128, SCA * 128), ml_dtypes.float8_e4m3)
        m1 = np.zeros((TPC, 128, 128), np.float16)
        m2 = np.zeros((TPC, 128, 128), np.float16) if has_b0 else None
        ia_arr = np.zeros((128, SA * 8), np.int16)
        ib_arr = np.zeros((128, SB * 8), np.int16)
        dk2 = np.full((128, 2 * (SA + SB)), 255.0, np.float16)
        dinvp = np.zeros((128, TPC), np.float32)
        acol = bcol = xcol = 0
        for k in range(TPC):
            hr, dl, spk, iam, ap_ = EB[c][k]
            ca, cca, ccb = CA[k], CCA[k], CCB[k]

            # agg0 stream: [128 nodes, ca slots, 128 f], pads -> zero row
            blk = np.full((128, ca), NPAD, np.int64)
            if len(dl):
                starts = np.concatenate(
                    [[0], np.flatnonzero(np.diff(dl)) + 1])
                lens = np.diff(np.concatenate([starts, [len(dl)]]))
                j_idx = np.arange(len(dl)) - np.repeat(starts, lens)
                blk[dl, j_idx] = spk
            xe[:, xcol * 128:(xcol + ca) * 128] = \
                x_pre_pos[blk.ravel()].reshape(128, ca * 128)
            xcol += ca

            # agg1 gather metadata (appended self loops excluded)
            ns = ~ap_
            hr_a, dl_a = hr[iam & ns], dl[iam & ns]
            hr_b, dl_b = hr[~iam & ns], dl[~iam & ns]
            iaw = np.zeros(cca * 128, np.int16)
            iaw[:len(hr_a)] = hr_a.astype(np.int16)
            ibw = np.zeros(ccb * 128, np.int16)
            ibw[:len(hr_b)] = hr_b.astype(np.int16)
            ia_arr[:, acol * 8:(acol + cca) * 8] = _wrap_idx16(iaw)
            ib_arr[:, bcol * 8:(bcol + ccb) * 8] = _wrap_idx16(ibw)

            # dst-row streams for on-chip one-hot build (255 = pad),
            # pair-duplicated so DVE is_equal runs in 2x mode.
            dka = np.full(cca * 128, 255, np.int64)
            dka[:len(dl_a)] = dl_a
            dka = dka.reshape(cca, 128).T.astype(np.float16)
            dk2[:, 2 * acol:2 * (acol + cca)] = \
                np.repeat(dka, 2, axis=1)
            dkb = np.full(ccb * 128, 255, np.int64)
            dkb[:len(dl_b)] = dl_b
            dkb = dkb.reshape(ccb, 128).T.astype(np.float16)
            dk2[:, 2 * (SA + bcol):2 * (SA + bcol + ccb)] = \
                np.repeat(dkb, 2, axis=1)
            acol += cca
            bcol += ccb

            nodes_pos = (k * NCORES + c) * 128 + np.arange(128)
            real = nodes_pos < N
            pn = perm[np.clip(nodes_pos, 0, N - 1)]
            dinvp[:, k] = dinv_pos[nodes_pos]
            m1k = drop_mask[pn] * (dinv[pn] ** 2)[:, None]
            m1k[~real] = 0.0
            m1[k] = m1k.astype(np.float16)
            if has_b0:
                m2k = drop_mask[pn] * b0[None, :] * dinv[pn][:, None]
                m2k[~real] = 0.0
                m2[k] = m2k.astype(np.float16)

        im = {"xe": xe, "m1": m1, "ia": ia_arr, "ib": ib_arr, "dk2": dk2,
              "dinvp": dinvp, "ident": ident_np, "ident82": ident8_np,
              "iota": iota_np,
              "w0t": np.ascontiguousarray(W0.T / sx).astype(np.float16),
              "w1t": np.ascontiguousarray(W1.T).astype(np.float16),
              "w2t": np.ascontiguousarray(W2.T).astype(np.float16)}
        if has_b0:
            im["m2"] = m2
        if has_b12:
            im["b1b"] = np.tile(b1, (128, 1))
            im["b2b"] = np.tile(b2, (128, 1))
        in_maps.append(im)

    nc = _build_kernel(CCA, CCB, CA, has_b0, has_b12)
    res = run_bass_kernel_spmd(
        nc, in_maps, core_ids=list(range(NCORES)),
        trace=(os.environ.get("KTRACE", "0") == "1"))
    kernel.last_result = res

    out1 = np.zeros((NPAD, 128), np.float32)
    out2 = np.zeros((NPAD, 128), np.float32)
    for c in range(NCORES):
        r1 = res.results[c]["o1"].reshape(NPC, 128).astype(np.float32)
        r2 = res.results[c]["o2"].reshape(NPC, 128).astype(np.float32)
        for k in range(TPC):
            t = k * NCORES + c
            out1[t * 128:(t + 1) * 128] = r1[k * 128:(k + 1) * 128]
            out2[t * 128:(t + 1) * 128] = r2[k * 128:(k + 1) * 128]
    return out1[pos], out2[pos]


# revision 38
# speedup vs baseline: 1.0291x; 1.0291x over previous
"""GCN encoder kernel for 8 Trainium2 NeuronCores (Bass/Tile, SPMD).

Strategy (dst-sharded graph parallel):
  - Nodes degree-sorted, padded to NPAD = 392 tiles of 128; tiles round-robin
    across the 8 cores (SPMD: one program, 8 in_maps).
  - Layer fusion: aggregation is linear, so each GCN layer is (aggregate)@W.T;
    layers 1 and 2 share ONE aggregation of h.
  - agg0 (over x): host expands x*dinv[src] into a padded per-(node,slot)
    fp8e3 stream (rms-normalized; the scale is folded into W0); device
    reduces with PE matmuls against an fp8 identity (PSUM scatter-add).
  - h is exchanged in FOUR AllGather slices (A1/A2 cover the first KA=18
    per-core tiles -> h_fullA, B1/B2 the last 31 -> h_fullB), each issued as
    soon as its h tiles exist, so agg1 pass 1 starts ~100us in, while phase A
    is still running.  Both gathered tensors have <32768 rows so int16
    gather indices need no range splitting.
  - agg1 (over h) runs as TWO PASSES over all tiles: pass 1 accumulates the
    h_fullA-half of every tile into f32 SBUF partials and is INTERLEAVED
    into the phase-A loop tail (PE alternates phase-A and pass-1 matmuls,
    and the serial SWDGE gather stream starts early); pass 2 adds the
    h_fullB half + the self-loop (one matmul against the SBUF-resident
    local h tile -- no gather) + partial, then applies W1/W2 and writes
    outputs.  Pass-1 gathers are pair-batched (2 tiles per SWDGE call) to
    amortize the ~1us fixed Q7 cost per dma_gather.
  - One-hot S matrices are built on-chip by DVE is_equal against a resident
    iota tile; the dst-row stream is PAIR-DUPLICATED (dk2) so every operand's
    innermost dim is packed stride-1 and the DVE 2x mode engages.
  - Weight matmuls run in f16; symmetric normalization is folded into
    host-side scales; outputs written f16 and upcast on host.
"""
import os
import sys

sys.path.insert(0, "/opt/trn_rl_repo")

import numpy as np

N, E, DIN, DH = 50000, 1600000, 128, 128
NCORES = 8
NPAD = ((N + 1023) // 1024) * 1024   # 50176 = 392 tiles of 128
TILES = NPAD // 128
TPC = TILES // NCORES                # positions (tiles) per core
NPC = TPC * 128                      # node rows per core
KA = 18                              # tiles in first AllGather half
KB = TPC - KA                        # tiles in second half (31)
NRA = NCORES * KA * 128              # 18432 rows in h_fullA (< 32768)
NRB = NCORES * KB * 128              # 31744 rows in h_fullB (< 32768)


def _wrap_idx16(a):
    """dma_gather index layout: idx i -> [i%16, i//16], replicated 8x."""
    n = len(a)
    w = np.zeros((16, n // 16), np.int16)
    w[np.arange(n) % 16, np.arange(n) // 16] = a
    return np.tile(w, (8, 1))


def _build_kernel(CCA, CCB, CA, has_b0, has_b12):
    """Build the SPMD Tile program. CCA/CCB/CA are per-position chunk
    counts (compile-time constants, shared by all cores)."""
    import concourse.bass as bass  # noqa: F401
    import concourse.tile as tile
    from concourse import bacc, mybir

    f32, f16, i16 = mybir.dt.float32, mybir.dt.float16, mybir.dt.int16
    f8 = mybir.dt.float8e4
    SA, SB = sum(CCA), sum(CCB)
    SCA = sum(CA)
    CAmax = max(CA)
    # gathers are pair-batched: one SWDGE call covers tiles (2j, 2j+1)
    PAIRS = [(2 * j, 2 * j + 1) if 2 * j + 1 < TPC else (2 * j,)
             for j in range((TPC + 1) // 2)]
    GW = max(max(sum(CCA[t] for t in pr) for pr in PAIRS),
             max(sum(CCB[t] for t in pr) for pr in PAIRS))
    CCmax = max(max(CCA), max(CCB))

    nc = bacc.Bacc(None, target_bir_lowering=False, debug=False,
                   num_swdge_queues=4)

    xe_d = nc.dram_tensor("xe", [128, SCA * 128], f8, kind="ExternalInput")
    m1_d = nc.dram_tensor("m1", [TPC, 128, 128], f16, kind="ExternalInput")
    ia_d = nc.dram_tensor("ia", [128, SA * 8], i16, kind="ExternalInput")
    ib_d = nc.dram_tensor("ib", [128, SB * 8], i16, kind="ExternalInput")
    dk_d = nc.dram_tensor("dk2", [128, 2 * (SA + SB)], f16,
                          kind="ExternalInput")
    dinv_d = nc.dram_tensor("dinvp", [128, TPC], f32, kind="ExternalInput")
    ident_d = nc.dram_tensor("ident", [128, 128], f16, kind="ExternalInput")
    ident8_d = nc.dram_tensor("ident82", [128, 256], f8, kind="ExternalInput")
    iota_d = nc.dram_tensor("iota", [128, 128], f16, kind="ExternalInput")
    w0_d = nc.dram_tensor("w0t", [128, 128], f16, kind="ExternalInput")
    w1_d = nc.dram_tensor("w1t", [128, 128], f16, kind="ExternalInput")
    w2_d = nc.dram_tensor("w2t", [128, 128], f16, kind="ExternalInput")
    if has_b0:
        m2_d = nc.dram_tensor("m2", [TPC, 128, 128], f16, kind="ExternalInput")
    if has_b12:
        b1_d = nc.dram_tensor("b1b", [128, 128], f32, kind="ExternalInput")
        b2_d = nc.dram_tensor("b2b", [128, 128], f32, kind="ExternalInput")
    o1_d = nc.dram_tensor("o1", [TPC, 128, 128], f16, kind="ExternalOutput")
    o2_d = nc.dram_tensor("o2", [TPC, 128, 128], f16, kind="ExternalOutput")

    offA = np.cumsum([0] + CCA).tolist()
    offB = np.cumsum([0] + CCB).tolist()
    GRP = [list(range(NCORES))]

    with tile.TileContext(nc) as tc:
        with (
            tc.tile_pool(name="const", bufs=1) as cpool,
            tc.tile_pool(name="acc", bufs=1) as apool,
            tc.tile_pool(name="dram", bufs=1, space="DRAM") as dpool,
            tc.tile_pool(name="g", bufs=5) as gpool,
            tc.tile_pool(name="s", bufs=2) as spool,
            tc.tile_pool(name="y2", bufs=2) as ypool2,
            tc.tile_pool(name="o", bufs=3) as opool,
            tc.tile_pool(name="psb", bufs=2, space="PSUM") as ppoolb,
            tc.tile_pool(name="psb2", bufs=1, space="PSUM") as ppoolb2,
        ):
            ident_sb = cpool.tile([128, 128], f16)
            nc.scalar.dma_start(ident_sb[:], ident_d[:])
            ident8_sb = cpool.tile([128, 256], f8)
            nc.scalar.dma_start(ident8_sb[:], ident8_d[:])
            iota_sb = cpool.tile([128, 128], f16)
            nc.scalar.dma_start(iota_sb[:], iota_d[:])
            w0_sb = cpool.tile([128, 128], f16)
            nc.scalar.dma_start(w0_sb[:], w0_d[:])
            w1_sb = cpool.tile([128, 128], f16)
            nc.scalar.dma_start(w1_sb[:], w1_d[:])
            w2_sb = cpool.tile([128, 128], f16)
            nc.scalar.dma_start(w2_sb[:], w2_d[:])
            dinv_sb = cpool.tile([128, TPC], f32)
            nc.scalar.dma_start(dinv_sb[:], dinv_d[:])
            ia_sb = cpool.tile([128, SA * 8], i16)
            ib_sb = cpool.tile([128, SB * 8], i16)
            dk_sb = cpool.tile([128, 2 * (SA + SB)], f16)
            if has_b12:
                b1_sb = cpool.tile([128, 128], f32)
                nc.scalar.dma_start(b1_sb[:], b1_d[:])
                b2_sb = cpool.tile([128, 128], f32)
                nc.scalar.dma_start(b2_sb[:], b2_d[:])

            h_sbA = apool.tile([128, KA, 128], f16)
            h_sbB = apool.tile([128, KB, 128], f16)
            part_sb = apool.tile([128, TPC, 128], f16)
            m1_sb = apool.tile([128, TPC, 128], f16)
            nc.scalar.dma_start(
                m1_sb[:], m1_d[:].rearrange("t p f -> p t f"))

            h_locA = dpool.tile([KA, 128, 128], f16)
            h_locB = dpool.tile([KB, 128, 128], f16)
            h_fullA = dpool.tile([NRA, 128], f16, addr_space="Shared")
            h_fullB = dpool.tile([NRB, 128], f16, addr_space="Shared")

            qctr = [0]

            def gather_pair(j, which):
                pr = PAIRS[j]
                CC = CCA if which == 0 else CCB
                cc = sum(CC[t] for t in pr)
                off = (offA if which == 0 else offB)[pr[0]]
                src = h_fullA if which == 0 else h_fullB
                idx = ia_sb if which == 0 else ib_sb
                G = gpool.tile([128, GW, 128], f16, tag="G")
                nc.gpsimd.dma_gather(G[:, 0:cc, :], src[:],
                                     idx[:, off * 8:(off + cc) * 8],
                                     cc * 128, cc * 128, 128,
                                     elem_step=128, single_packet=False,
                                     queue_num=qctr[0] % 4)
                qctr[0] += 1
                return G

            def build_S(k, which):
                cc = (CCA if which == 0 else CCB)[k]
                col = (offA[k] if which == 0 else SA + offB[k])
                S = spool.tile([128, CCmax, 128], f16, tag="S")
                iota4 = iota_sb[:].rearrange(
                    "p (o f g) -> p o f g", o=1, g=2).broadcast_to(
                    [128, cc, 64, 2])
                dk4 = dk_sb[:, 2 * col:2 * (col + cc)].rearrange(
                    "p (c o g) -> p c o g", o=1, g=2).broadcast_to(
                    [128, cc, 64, 2])
                S4 = S[:, 0:cc, :].rearrange("p c (f g) -> p c f g", g=2)
                nc.vector.tensor_tensor(S4, iota4, dk4,
                                        mybir.AluOpType.is_equal)
                return S

            PRE1 = 2                 # prefetched pass-1 pair gathers
            PRE2 = 2                 # prefetched pass-2 pair gathers
            BCOLL_AT = 4             # pass-1 pair after which B-coll is issued
            Gp = [None] * len(PAIRS)
            np_ctr = [0]

            def process_pair(j):
                if j + PRE1 < len(PAIRS):
                    Gp[j + PRE1] = gather_pair(j + PRE1, 0)
                G = Gp[j]
                Gp[j] = None
                goff = 0
                for t in PAIRS[j]:
                    cca = CCA[t]
                    S = build_S(t, 0)
                    ps_b = ppoolb.tile([128, 128], f32, tag="pb")
                    for c in range(cca):
                        nc.tensor.matmul(ps_b[:], G[:, goff + c, :],
                                         S[:, c, :],
                                         start=(c == 0), stop=(c == cca - 1))
                    nc.scalar.copy(part_sb[:, t, :], ps_b[:])
                    goff += cca

            # ---------------- Phase A: agg0 + h (pass 1 interleaved) ------
            with (
                tc.tile_pool(name="xe", bufs=3) as xpool,
                tc.tile_pool(name="meta", bufs=3) as mpool,
                tc.tile_pool(name="y", bufs=3) as ypool,
                tc.tile_pool(name="ps", bufs=3, space="PSUM") as ppool,
                tc.tile_pool(name="ps2", bufs=2, space="PSUM") as ppool2,
            ):
                # the W0 matmul of tile k-1 is emitted AFTER tile k's psum
                # accumulation: the PE then never waits on the scalar copy,
                # stays continuously busy, and ramps to the full pstate.
                def w0_tail(k, y_sb):
                    ps_h = ppool2.tile([128, 128], f32, tag="ph")
                    nc.tensor.matmul(ps_h[:], y_sb[:], w0_sb[:],
                                     start=True, stop=True)  # [n, f2]
                    tmp = ypool.tile([128, 128], f32, tag="tmp")
                    nc.vector.tensor_tensor(tmp[:], ps_h[:], m1_sb[:, k, :],
                                            mybir.AluOpType.mult)
                    if has_b0:
                        m2_sb = mpool.tile([128, 128], f16, tag="m2")
                        nc.sync.dma_start(m2_sb[:], m2_d[k])
                        nc.vector.tensor_tensor(tmp[:], tmp[:], m2_sb[:],
                                                mybir.AluOpType.add)
                    if k < KA:
                        nc.scalar.activation(h_sbA[:, k, :], tmp[:],
                                             mybir.ActivationFunctionType.Relu)
                    else:
                        nc.scalar.activation(h_sbB[:, k - KA, :], tmp[:],
                                             mybir.ActivationFunctionType.Relu)

                acol = 0
                pend = [None, None]          # (k, y_sb) awaiting its W0 tail
                for k in range(TPC):
                    ca = CA[k]               # even by construction
                    ca2 = ca // 2
                    gt = xpool.tile([128, CAmax // 2, 256], f8, tag="gt")
                    nc.sync.dma_start(
                        gt[:, 0:ca2, :].rearrange("p c f -> p (c f)"),
                        xe_d[:, acol * 128:(acol + ca) * 128])
                    acol += ca

                    # fp8e4 DoubleRow: each matmul contracts TWO slots
                    # (lhsT = [slot2j | slot2j+1], rhs = [I | I]), halving
                    # the PE instruction count -- phase A is issue-bound.
                    ps_a = ppool.tile([128, 128], f32, tag="pa")
                    id2 = ident8_sb[:].rearrange("p (two f) -> p two f", two=2)
                    for j in range(ca2):
                        nc.tensor.matmul(ps_a[:],
                                         gt[:, j, :].rearrange(
                                             "p (two f) -> p two f", two=2),
                                         id2,
                                         start=(j == 0), stop=(j == ca2 - 1),
                                         perf_mode=mybir.MatmulPerfMode.DoubleRow)
                    y_sb = ypool.tile([128, 128], f16, tag="y0")
                    nc.scalar.copy(y_sb[:], ps_a[:])        # [f, d] raw sums
                    if pend[0] is not None:
                        w0_tail(*pend)
                    pend = [k, y_sb]

                    if k == 1:
                        nc.scalar.dma_start(ia_sb[:], ia_d[:])
                        nc.scalar.dma_start(ib_sb[:], ib_d[:])
                        nc.scalar.dma_start(dk_sb[:], dk_d[:])
                    if k == KA:
                        # tile KA-1's tail ran above, so h_sbA is complete
                        nc.gpsimd.dma_start(
                            h_locA[:].rearrange("t p f -> p t f"), h_sbA[:])
                        nc.gpsimd.collective_compute(
                            "AllGather", mybir.AluOpType.bypass,
                            replica_groups=GRP,
                            ins=[h_locA[:]], outs=[h_fullA[:]],
                        )
                w0_tail(*pend)
                # B-coll FIRST on the Pool queue: its h_locB wait gates the
                # gather stream until the phase-A xe stream has drained the
                # DMA engines (otherwise gathers starve it and stall the h
                # tail), and the collective itself runs on CC/RDMA overlapped
                # with pass-1 gathers.  Pool issues the DMA (Sync would
                # deadlock behind gt loads).
                nc.gpsimd.dma_start(
                    h_locB[:].rearrange("t p f -> p t f"), h_sbB[:])
                nc.gpsimd.collective_compute(
                    "AllGather", mybir.AluOpType.bypass,
                    replica_groups=GRP,
                    ins=[h_locB[:]], outs=[h_fullB[:]],
                )
                for jj in range(PRE1):
                    Gp[jj] = gather_pair(jj, 0)
                while np_ctr[0] < len(PAIRS):
                    process_pair(np_ctr[0])
                    np_ctr[0] += 1

            # ---------------- Phase B pass 2: B-half + outputs ------------
            Gp2 = [None] * len(PAIRS)
            for jj in range(min(PRE2, len(PAIRS))):
                Gp2[jj] = gather_pair(jj, 1)
            for j in range(len(PAIRS)):
                if j + PRE2 < len(PAIRS):
                    Gp2[j + PRE2] = gather_pair(j + PRE2, 1)
                G = Gp2[j]
                Gp2[j] = None
                goff = 0
                for k in PAIRS[j]:
                    ccb = CCB[k]
                    S = build_S(k, 1)
                    h_self = (h_sbA[:, k, :] if k < KA
                              else h_sbB[:, k - KA, :])
                    ps_b = ppoolb.tile([128, 128], f32, tag="pb")
                    # self-loop: h of this tile is SBUF-resident, no gather
                    nc.tensor.matmul(ps_b[:], h_self, ident_sb[:],
                                     start=True, stop=False)
                    for c in range(ccb):
                        nc.tensor.matmul(ps_b[:], G[:, goff + c, :],
                                         S[:, c, :],
                                         start=False, stop=(c == ccb - 1))
                    goff += ccb
                    y2 = ypool2.tile([128, 128], f16, tag="y2")
                    nc.vector.tensor_tensor(y2[:], ps_b[:], part_sb[:, k, :],
                                            mybir.AluOpType.add)

                    ps_o1 = ppoolb2.tile([128, 128], f32, tag="po")
                    nc.tensor.matmul(ps_o1[:], y2[:], w1_sb[:],
                                     start=True, stop=True)
                    o1t = opool.tile([128, 128], f16, tag="o1")
                    nc.scalar.activation(o1t[:], ps_o1[:],
                                         mybir.ActivationFunctionType.Copy,
                                         scale=dinv_sb[:, k:k + 1])
                    ps_o2 = ppoolb2.tile([128, 128], f32, tag="po")
                    nc.tensor.matmul(ps_o2[:], y2[:], w2_sb[:],
                                     start=True, stop=True)
                    o2t = opool.tile([128, 128], f16, tag="o2")
                    nc.scalar.activation(o2t[:], ps_o2[:],
                                         mybir.ActivationFunctionType.Copy,
                                         scale=dinv_sb[:, k:k + 1])
                    if has_b12:
                        nc.vector.tensor_tensor(o1t[:], o1t[:], b1_sb[:],
                                                mybir.AluOpType.add)
                        nc.vector.tensor_tensor(o2t[:], o2t[:], b2_sb[:],
                                                mybir.AluOpType.add)
                    nc.sync.dma_start(o1_d[k], o1t[:])
                    nc.sync.dma_start(o2_d[k], o2t[:])

    nc.compile()
    return nc


def kernel(x, edge_index, drop_mask, W0, b0, W1, b1, W2, b2, **_):
    import ml_dtypes
    from concourse.bass_utils import run_bass_kernel_spmd

    x = np.asarray(x, np.float32)
    edge_index = np.asarray(edge_index)
    drop_mask = np.asarray(drop_mask, np.float32)
    W0, W1, W2 = (np.asarray(w, np.float32) for w in (W0, W1, W2))
    b0, b1, b2 = (np.asarray(b, np.float32) for b in (b0, b1, b2))
    src0, dst0 = edge_index[0].astype(np.int64), edge_index[1].astype(np.int64)

    # ---- normalization / permutation (host: index-side preprocessing) ----
    deg = np.bincount(dst0, minlength=N).astype(np.float32) + 1.0
    dinv = 1.0 / np.sqrt(deg)

    perm = np.argsort(-deg, kind="stable")           # position -> node id
    pos = np.empty(N, np.int64)                      # node id -> position
    pos[perm] = np.arange(N)

    # self loops as ordinary edges; the appended ones (apnd) are handled in
    # phase B by a local matmul instead of gathers.
    src_a = np.concatenate([src0, np.arange(N)])
    dst_a = np.concatenate([dst0, np.arange(N)])
    apnd = np.concatenate([np.zeros(E, bool), np.ones(N, bool)])
    sp = pos[src_a]
    dp = pos[dst_a]

    # h storage row of a source position, split across the two AllGathers:
    #   tile t = p//128 -> core t%8, per-core tile index t//8
    #   first KA per-core tiles -> h_fullA, rest -> h_fullB
    st = sp // 128
    sk = st // NCORES
    in_a = sk < KA
    hrow = np.where(
        in_a,
        (st % NCORES) * (KA * 128) + sk * 128 + (sp % 128),
        (st % NCORES) * (KB * 128) + (sk - KA) * 128 + (sp % 128))

    tile_of = dp // 128
    core_of = tile_of % NCORES
    kpos_of = tile_of // NCORES

    order = np.lexsort((sp, dp))
    sp, dp = sp[order], dp[order]
    core_of, kpos_of = core_of[order], kpos_of[order]
    hrow, in_a, apnd = hrow[order], in_a[order], apnd[order]
    dloc = dp % 128

    # fp8e3 x-stream: rms-normalize so values sit in e3m4's sweet spot;
    # the scale is undone inside W0 (aggregation is linear).
    x_pre = x * dinv[:, None]
    sx = float(1.0 / np.sqrt((x_pre ** 2).mean()))
    x_pre_pos = np.zeros((NPAD + 1, 128), ml_dtypes.float8_e4m3)
    x_pre_pos[pos] = (x_pre * sx).astype(ml_dtypes.float8_e4m3)
    dinv_pos = np.zeros(NPAD, np.float32)
    dinv_pos[pos] = dinv

    # ---- per-(core, position) edge groups ----
    EB = [[None] * TPC for _ in range(NCORES)]
    for c in range(NCORES):
        mc = core_of == c
        spc, kc, dl = sp[mc], kpos_of[mc], dloc[mc]
        hr, ia, ap_ = hrow[mc], in_a[mc], apnd[mc]
        for k in range(TPC):
            mk = kc == k
            EB[c][k] = (hr[mk], dl[mk], spc[mk], ia[mk], ap_[mk])

    # per-position chunk counts (max over cores -> same program everywhere)
    # CA: phase-A slots (all edges incl. appended self loops)
    # CCA/CCB: phase-B gather chunks (appended self loops excluded)
    CCA, CCB, CA = [], [], []
    for k in range(TPC):
        cca = ccb = ca = 0
        for c in range(NCORES):
            hr, dl, _, ia, ap_ = EB[c][k]
            ns = ~ap_
            na = int((ia & ns).sum())
            nb = int((~ia & ns).sum())
            cca = max(cca, -(-na // 128))
            ccb = max(ccb, -(-nb // 128))
            if len(dl):
                ca = max(ca, int(np.bincount(dl, minlength=128).max()))
        CCA.append(max(cca, 1))
        CCB.append(max(ccb, 1))
        ca = max(ca, 2)
        CA.append(ca + (ca & 1))             # even, for DoubleRow pairs
    SCA, SA, SB = sum(CA), sum(CCA), sum(CCB)

    ident_np = np.eye(128, dtype=np.float16)
    ident8_np = np.tile(np.eye(128), (1, 2)).astype(ml_dtypes.float8_e4m3)
    iota_np = np.tile(np.arange(128, dtype=np.float16), (128, 1))
    has_b0 = bool(np.any(b0))
    has_b12 = bool(np.any(b1)) or bool(np.any(b2))

    in_maps = []
    for c in range(NCORES):
        xe = np.zeros((128, SCA * 128), ml_dtypes.float8_e4m3)
        m1 = np.zeros((TPC, 128, 128), np.float16)
        m2 = np.zeros((TPC, 128, 128), np.float16) if has_b0 else None
        ia_arr = np.zeros((128, SA * 8), np.int16)
        ib_arr = np.zeros((128, SB * 8), np.int16)
        dk2 = np.full((128, 2 * (SA + SB)), 255.0, np.float16)
        dinvp = np.zeros((128, TPC), np.float32)
        acol = bcol = xcol = 0
        for k in range(TPC):
            hr, dl, spk, iam, ap_ = EB[c][k]
            ca, cca, ccb = CA[k], CCA[k], CCB[k]

            # agg0 stream: [128 nodes, ca slots, 128 f], pads -> zero row
            blk = np.full((128, ca), NPAD, np.int64)
            if len(dl):
                starts = np.concatenate(
                    [[0], np.flatnonzero(np.diff(dl)) + 1])
                lens = np.diff(np.concatenate([starts, [len(dl)]]))
                j_idx = np.arange(len(dl)) - np.repeat(starts, lens)
                blk[dl, j_idx] = spk
            xe[:, xcol * 128:(xcol + ca) * 128] = \
                x_pre_pos[blk.ravel()].reshape(128, ca * 128)
            xcol += ca

            # agg1 gather metadata (appended self loops excluded)
            ns = ~ap_
            hr_a, dl_a = hr[iam & ns], dl[iam & ns]
            hr_b, dl_b = hr[~iam & ns], dl[~iam & ns]
            iaw = np.zeros(cca * 128, np.int16)
            iaw[:len(hr_a)] = hr_a.astype(np.int16)
            ibw = np.zeros(ccb * 128, np.int16)
            ibw[:len(hr_b)] = hr_b.astype(np.int16)
            ia_arr[:, acol * 8:(acol + cca) * 8] = _wrap_idx16(iaw)
            ib_arr[:, bcol * 8:(bcol + ccb) * 8] = _wrap_idx16(ibw)

            # dst-row streams for on-chip one-hot build (255 = pad),
            # pair-duplicated so DVE is_equal runs in 2x mode.
            dka = np.full(cca * 128, 255, np.int64)
            dka[:len(dl_a)] = dl_a
            dka = dka.reshape(cca, 128).T.astype(np.float16)
            dk2[:, 2 * acol:2 * (acol + cca)] = \
                np.repeat(dka, 2, axis=1)
            dkb = np.full(ccb * 128, 255, np.int64)
            dkb[:len(dl_b)] = dl_b
            dkb = dkb.reshape(ccb, 128).T.astype(np.float16)
            dk2[:, 2 * (SA + bcol):2 * (SA + bcol + ccb)] = \
                np.repeat(dkb, 2, axis=1)
            acol += cca
            bcol += ccb

            nodes_pos = (k * NCORES + c) * 128 + np.arange(128)
            real = nodes_pos < N
            pn = perm[np.clip(nodes_pos, 0, N - 1)]
            dinvp[:, k] = dinv_pos[nodes_pos]
            m1k = drop_mask[pn] * (dinv[pn] ** 2)[:, None]
            m1k[~real] = 0.0
            m1[k] = m1k.astype(np.float16)
            if has_b0:
                m2k = drop_mask[pn] * b0[None, :] * dinv[pn][:, None]
                m2k[~real] = 0.0
                m2[k] = m2k.astype(np.float16)

        im = {"xe": xe, "m1": m1, "ia": ia_arr, "ib": ib_arr, "dk2": dk2,
              "dinvp": dinvp, "ident": ident_np, "ident82": ident8_np,
              "iota": iota_np,
              "w0t": np.ascontiguousarray(W0.T / sx).astype(np.float16),
              "w1t": np.ascontiguousarray(W1.T).astype(np.float16),
              "w2t": np.ascontiguousarray(W2.T).astype(np.float16)}
        if has_b0:
            im["m2"] = m2
        if has_b12:
            im["b1b"] = np.tile(b1, (128, 1))
            im["b2b"] = np.tile(b2, (128, 1))
        in_maps.append(im)

    nc = _build_kernel(CCA, CCB, CA, has_b0, has_b12)
    res = run_bass_kernel_spmd(
        nc, in_maps, core_ids=list(range(NCORES)),
        trace=(os.environ.get("KTRACE", "0") == "1"))
    kernel.last_result = res

    out1 = np.zeros((NPAD, 128), np.float32)
    out2 = np.zeros((NPAD, 128), np.float32)
    for c in range(NCORES):
        r1 = res.results[c]["o1"].reshape(NPC, 128).astype(np.float32)
        r2 = res.results[c]["o2"].reshape(NPC, 128).astype(np.float32)
        for k in range(TPC):
            t = k * NCORES + c
            out1[t * 128:(t + 1) * 128] = r1[k * 128:(k + 1) * 128]
            out2[t * 128:(t + 1) * 128] = r2[k * 128:(k + 1) * 128]
    return out1[pos], out2[pos]
